# revision 4
# baseline (speedup 1.0000x reference)
"""GroupLinear (soft MoE routing) Trainium2 Bass kernel.

Computes out[b,o] = sum_j g[b,j] * (x[b,:] @ W[j,:,:])[o] + (g @ bias_p)[b,o]
for B=16384, G=16, DIN=DOUT=512, fp32.

Sharding: data-parallel over batch across 8 NeuronCores (2048 rows/core);
weight + bias replicated. Per core:
  - W resident in SBUF as 64 tiles [128, 512] (16 MB).
  - per 128-row batch tile: transpose x on PE (4x [128,128] via identity),
    bias term via one K=16 matmul (gT @ bias), then for each group j:
    Y_j = x @ W_j accumulated over 4 K-chunks in PSUM (float32r matmuls,
    1 cycle/row at N=512), scaled by g[:,j] on ScalarE (per-partition
    scale), accumulated on VectorE.
"""

import numpy as np

import concourse.bass as bass
import concourse.tile as tile
from concourse import bacc, mybir
from concourse.bass_utils import run_bass_kernel_spmd
from concourse.masks import make_identity

B, G, DIN, DOUT = 16384, 16, 512, 512
NCORES = 8
BC = B // NCORES          # rows per core
P = 128                   # partitions
NBT = BC // P             # batch tiles per core
KC = DIN // P             # contraction chunks (4)

F32 = mybir.dt.float32
F32R = mybir.dt.float32r
BF16 = mybir.dt.bfloat16

# matmul input dtype: "f32r" (fp32 data, reduced-precision PE mode) or "bf16"
MM_DTYPE = "f32r"


def _emit(nc, tc, out_ap, x_ap, g_ap, w_ap, bias_ap, ctx):
    f32r = MM_DTYPE == "f32r"

    const_pool = ctx.enter_context(tc.tile_pool(name="const", bufs=1))
    wpool = ctx.enter_context(tc.tile_pool(name="wpool", bufs=1))
    xpool = ctx.enter_context(tc.tile_pool(name="xpool", bufs=3))
    gpool = ctx.enter_context(tc.tile_pool(name="gpool", bufs=3))
    xtpool = ctx.enter_context(tc.tile_pool(name="xtpool", bufs=2))
    gtpool = ctx.enter_context(tc.tile_pool(name="gtpool", bufs=2))
    accpool = ctx.enter_context(tc.tile_pool(name="accpool", bufs=2))
    tmppool = ctx.enter_context(tc.tile_pool(name="tmppool", bufs=3))
    ps_y = ctx.enter_context(tc.tile_pool(name="ps_y", bufs=4, space="PSUM"))
    ps_yb = ctx.enter_context(tc.tile_pool(name="ps_yb", bufs=2, space="PSUM"))
    ps_t = ctx.enter_context(tc.tile_pool(name="ps_t", bufs=2, space="PSUM"))

    mm_dt = F32R if f32r else BF16

    # identity for PE transposes
    ident = const_pool.tile([P, P], F32, name="ident")
    make_identity(nc, ident)

    # bias resident [16, 512], converted to the matmul dtype
    bias_st = const_pool.tile([G, DOUT], F32, name="bias_st")
    nc.sync.dma_start(bias_st[:], bias_ap[:, :])
    bias_sb = const_pool.tile([G, DOUT], mm_dt, name="bias_sb")
    nc.vector.tensor_copy(bias_sb[:], bias_st[:])

    # first batch tile's x/g before the big weight load so its DMA isn't
    # queued behind 16 MB of weights
    xt0 = xpool.tile([P, DIN], F32, tag="xt", name="xt0")
    nc.sync.dma_start(xt0[:], x_ap[0:P, :])
    gt0 = gpool.tile([P, G], F32, tag="gt", name="gt0")
    nc.sync.dma_start(gt0[:], g_ap[0:P, :])

    # resident weights: w_sb[:, (j*KC+ic)*DOUT : ...] = W[j, ic*128:(ic+1)*P, :]
    # staged as fp32 from DRAM, converted (rounded) to mm dtype on DVE —
    # the FP32r/BF16 matmul inputs must be produced by a compute engine.
    w_sb = wpool.tile([P, G * KC * DOUT], mm_dt, name="w_sb")
    wstage = ctx.enter_context(tc.tile_pool(name="wstage", bufs=4))
    for j in range(G):
        for ic in range(KC):
            wst = wstage.tile([P, DOUT], F32, tag="wst", name="wst")
            nc.sync.dma_start(wst[:], w_ap[j, ic * P:(ic + 1) * P, :])
            nc.vector.tensor_copy(
                w_sb[:, (j * KC + ic) * DOUT:(j * KC + ic + 1) * DOUT], wst[:]
            )

    for bt in range(NBT):
        if bt == 0:
            xt, gt = xt0, gt0
        else:
            xt = xpool.tile([P, DIN], F32, tag="xt", name="xt")
            nc.sync.dma_start(xt[:], x_ap[bt * P:(bt + 1) * P, :])
            gt = gpool.tile([P, G], F32, tag="gt", name="gt")
            nc.sync.dma_start(gt[:], g_ap[bt * P:(bt + 1) * P, :])

        # transpose x tile: xT[:, ic*128:...] = xt[:, ic*128:...].T
        xT = xtpool.tile([P, DIN], mm_dt, tag="xT", name="xT")
        for ic in range(KC):
            tps = ps_t.tile([P, P], F32, tag="tps", name="tps")
            nc.tensor.transpose(tps[:], xt[:, ic * P:(ic + 1) * P], ident[:])
            nc.vector.tensor_copy(xT[:, ic * P:(ic + 1) * P], tps[:])

        # gT for the bias matmul
        gps = ps_t.tile([G, P], F32, tag="tps", name="gps")
        nc.tensor.transpose(gps[:], gt[:], ident[:])
        gT = gtpool.tile([G, P], mm_dt, tag="gT", name="gT")
        nc.vector.tensor_copy(gT[:], gps[:])

        # bias term: yb = g_tile @ bias  (K=16)
        yb = ps_yb.tile([P, DOUT], F32, tag="yb", name="yb")
        nc.tensor.matmul(yb[:], gT[:], bias_sb[:], start=True, stop=True)

        acc = accpool.tile([P, DOUT], F32, tag="acc", name="acc")
        for j in range(G):
            y = ps_y.tile([P, DOUT], F32, tag="y", name="y")
            for ic in range(KC):
                nc.tensor.matmul(
                    y[:],
                    xT[:, ic * P:(ic + 1) * P],
                    w_sb[:, (j * KC + ic) * DOUT:(j * KC + ic + 1) * DOUT],
                    start=(ic == 0),
                    stop=(ic == KC - 1),
                )
            if j == 0:
                # acc = g[:,0] * Y_0   (ScalarE, per-partition scale)
                nc.scalar.mul(acc[:], y[:], gt[:, 0:1])
            else:
                tmp = tmppool.tile([P, DOUT], F32, tag="tmp", name="tmp")
                nc.scalar.mul(tmp[:], y[:], gt[:, j:j + 1])
                nc.vector.tensor_add(acc[:], acc[:], tmp[:])
        # add bias term from PSUM
        nc.vector.tensor_add(acc[:], acc[:], yb[:])

        nc.sync.dma_start(out_ap[bt * P:(bt + 1) * P, :], acc[:])


def _build():
    nc = bacc.Bacc("TRN2", target_bir_lowering=False, debug=False)
    x_ap = nc.dram_tensor("x", [BC, DIN], F32, kind="ExternalInput").ap()
    g_ap = nc.dram_tensor("g", [BC, G], F32, kind="ExternalInput").ap()
    w_ap = nc.dram_tensor("weight", [G, DIN, DOUT], F32, kind="ExternalInput").ap()
    bias_ap = nc.dram_tensor("bias_p", [G, DOUT], F32, kind="ExternalInput").ap()
    out_ap = nc.dram_tensor("out", [BC, DOUT], F32, kind="ExternalOutput").ap()

    from contextlib import ExitStack

    with tile.TileContext(nc) as tc:
        with ExitStack() as ctx:
            _emit(nc, tc, out_ap, x_ap, g_ap, w_ap, bias_ap, ctx)
    nc.compile()
    return nc


_NC = None
last_result = None


def kernel(x, g, weight, bias_p):
    global _NC, last_result
    if _NC is None:
        _NC = _build()

    x = np.ascontiguousarray(np.asarray(x, dtype=np.float32))
    g = np.ascontiguousarray(np.asarray(g, dtype=np.float32))
    weight = np.ascontiguousarray(np.asarray(weight, dtype=np.float32))
    bias_p = np.ascontiguousarray(np.asarray(bias_p, dtype=np.float32))

    in_maps = [
        {
            "x": x[c * BC:(c + 1) * BC],
            "g": g[c * BC:(c + 1) * BC],
            "weight": weight,
            "bias_p": bias_p,
        }
        for c in range(NCORES)
    ]
    res = run_bass_kernel_spmd(_NC, in_maps, core_ids=list(range(NCORES)))
    last_result = res
    return np.concatenate([r["out"] for r in res.results], axis=0)
